# revision 3
# baseline (speedup 1.0000x reference)
"""LIF spike (vanilla) Trainium2 kernel.

Reference recurrence over leading time dim T (per element):
    u_t = TAU * u_{t-1} * (1 - o_{t-1}) + x_t
    o_t = (u_t - VTH > 0) ? 1.0 : 0.0

Decomposed into 3 DVE ops per time step on carried state c = u * (u <= VTH):
    S1: u = (c mult TAU) add x_t        (scalar_tensor_tensor, skipped at t=0)
    S2: o = (u is_gt VTH)               (tensor_scalar, fp32 2x mode)
    S3: c = (u is_le VTH) mult u        (scalar_tensor_tensor, skipped at t=T-1)

Sharding: pure data parallel over batch dim B=64 -> 8 cores x 8 batches.
Per core: [T=8, N=1048576] f32 in, same out. Memory-bound: ~67MB HBM
traffic/core at ~360GB/s => ~190us roofline.
"""

import numpy as np

T = 8
B = 64
C = 128
H = 32
W = 32
NCORES = 8
BS = B // NCORES            # batches per core
N = BS * C * H * W          # 1,048,576 elements per time step per core
P = 128                     # SBUF partitions
F = 4096                    # tile free-dim (tile = [128, 4096] f32 = 2MB)
NCHUNK = N // (P * F)       # spatial chunks per core
TAU = 0.5
VTH = 0.99999


def _build(nt=T, nchunk=NCHUNK, fdim=F, xb=3, ob=3, ub=2, cb=2):
    import concourse.bacc as bacc
    import concourse.mybir as mybir
    import concourse.tile as tile

    f32 = mybir.dt.float32
    alu = mybir.AluOpType
    nc = bacc.Bacc("TRN2", target_bir_lowering=False)
    x = nc.dram_tensor("x", [nt, nchunk, P, fdim], f32, kind="ExternalInput")
    o = nc.dram_tensor("o", [nt, nchunk, P, fdim], f32, kind="ExternalOutput")
    with tile.TileContext(nc) as tc:
        with (
            tc.tile_pool(name="xp", bufs=xb) as xp,
            tc.tile_pool(name="opool", bufs=ob) as opl,
            tc.tile_pool(name="up", bufs=ub) as up,
            tc.tile_pool(name="cp", bufs=cb) as cp,
        ):
            for i in range(nchunk):
                ct = None
                for t in range(nt):
                    xt = xp.tile([P, fdim], f32)
                    nc.sync.dma_start(xt[:], x[t, i])
                    if t == 0:
                        u = xt
                    else:
                        u = up.tile([P, fdim], f32)
                        nc.vector.scalar_tensor_tensor(
                            u[:], ct[:], TAU, xt[:], alu.mult, alu.add
                        )
                    ot = opl.tile([P, fdim], f32)
                    nc.vector.tensor_scalar(ot[:], u[:], VTH, None, alu.is_gt)
                    nc.sync.dma_start(o[t, i], ot[:])
                    if t < nt - 1:
                        ct = cp.tile([P, fdim], f32)
                        nc.vector.scalar_tensor_tensor(
                            ct[:], u[:], VTH, u[:], alu.is_le, alu.mult
                        )
    nc.finalize()
    return nc


def kernel(x):
    x = np.ascontiguousarray(np.asarray(x, dtype=np.float32))
    assert x.shape == (T, B, C, H, W), x.shape
    from concourse.bass_utils import run_bass_kernel_spmd

    nc = _build()
    in_maps = []
    for i in range(NCORES):
        s = np.ascontiguousarray(x[:, i * BS : (i + 1) * BS])
        in_maps.append({"x": s.reshape(T, NCHUNK, P, F)})
    res = run_bass_kernel_spmd(nc, in_maps, core_ids=list(range(NCORES)))
    out = np.empty((T, B, C, H, W), np.float32)
    for i, r in enumerate(res.results):
        out[:, i * BS : (i + 1) * BS] = np.asarray(r["o"]).reshape(T, BS, C, H, W)
    return out
